# revision 14
# baseline (speedup 1.0000x reference)
"""EdgeConv (PyG, aggr='max') Trainium2 kernel, 8-core SPMD.

Math: out_i = max_{e: dst(e)=i} relu(x_i @ W1.T + (x_src(e) - x_i) @ W2.T + b)
with W = [W1 | W2].  Rewriting:
    msg_e = relu(A_i + g_src(e)),  A = x @ (W1-W2).T + b,  g = x @ W2.T
Since A_i is constant within segment i and relu is monotone:
    out_i = relu(A_i + max_e g_src(e))
The reference's dst is repeat(arange(N), DEG) (fixed-degree kNN-style graph),
so segments are 16 consecutive edges; segment-max becomes a grouped reduce.

Two SPMD launches on 8 cores:
  L1 (node-parallel): per-core 6250-node shard computes A and g (row-major
     f32) via PE transpose of x tiles + two matmuls.
  L2 (edge-parallel): per-core 100k-edge shard bulk-gathers g rows by src
     with non-transpose dma_gather (256B-row descriptors). int16 index range
     is handled by splitting the table into lo/hi halves whose row 0 is a
     -3e38 sentinel; positions belonging to the other half (and node
     padding) use index 0.  A host-side edge permutation lands node n's 16
     rows at partition n%128, free slots 16*(n//128)+k, so the segment max
     is a contiguous free-dim grouped reduce and the result is row-major.
"""

import numpy as np

N_NODES = 50000
DEG = 16
C = 64
N_CORES = 8
NSH = N_NODES // N_CORES  # 6250 nodes per core
P = 128
TCH = 2  # node-tiles per L2 chunk
CHUNK = TCH * P  # 256 nodes per L2 chunk
NSH_PAD = 6400  # 25 chunks * 256; 50 tiles * 128
NT = NSH_PAD // P  # 50
NCHUNKS = NSH_PAD // CHUNK  # 25
NI = CHUNK * DEG  # 4096 gather positions per chunk
SPLIT = 25000  # table split for int16 indices
SENT = -3.0e38

_cache = {}


def _build_dense():
    import concourse.bacc as bacc
    import concourse.mybir as mybir
    from concourse.tile import TileContext
    from concourse.masks import make_identity

    nc = bacc.Bacc("TRN2", target_bir_lowering=False, debug=False)
    f32 = mybir.dt.float32
    xs = nc.dram_tensor("xs", [NSH_PAD, C], f32, kind="ExternalInput")
    w = nc.dram_tensor("w", [C, 2 * C], f32, kind="ExternalInput")
    bb = nc.dram_tensor("bb", [P, C], f32, kind="ExternalInput")
    gout = nc.dram_tensor("gout", [NSH_PAD, C], f32, kind="ExternalOutput")
    aout = nc.dram_tensor("aout", [NSH_PAD, C], f32, kind="ExternalOutput")

    with TileContext(nc) as tc:
        with (
            tc.tile_pool(name="const", bufs=1) as cpool,
            tc.tile_pool(name="sbuf", bufs=4) as pool,
            tc.tile_pool(name="psum", bufs=2, space="PSUM") as psum,
        ):
            ident = cpool.tile([P, P], f32)
            make_identity(nc, ident[:])
            w_sb = cpool.tile([C, 2 * C], f32)
            nc.sync.dma_start(out=w_sb[:], in_=w[:])
            bb_sb = cpool.tile([P, C], f32)
            nc.sync.dma_start(out=bb_sb[:], in_=bb[:])

            # wcat = [V1.T | V2.T] ([64, 128]): V1=W1-W2, V2=W2.
            t1 = psum.tile([C, C], f32, tag="wt")
            t2 = psum.tile([C, C], f32, tag="wt")
            nc.tensor.transpose(out=t1[:], in_=w_sb[:, 0:C], identity=ident[:C, :C])
            nc.tensor.transpose(out=t2[:], in_=w_sb[:, C : 2 * C], identity=ident[:C, :C])
            wcat = cpool.tile([C, 2 * C], f32)
            nc.vector.tensor_copy(out=wcat[:, C : 2 * C], in_=t2[:])
            nc.vector.tensor_sub(out=wcat[:, 0:C], in0=t1[:], in1=wcat[:, C : 2 * C])

            G = 5  # node-tiles per supertile (NT == 50 == 10 * G)
            for st in range(NT // G):
                rows = slice(st * G * P, (st + 1) * G * P)
                xt = pool.tile([P, G, C], f32, tag="xt")
                nc.sync.dma_start(
                    out=xt[:], in_=xs[rows, :].rearrange("(g p) c -> p g c", p=P)
                )
                g_sup = pool.tile([P, G, C], f32, tag="g")
                a_sup = pool.tile([P, G, C], f32, tag="a")
                for g in range(G):
                    xT_ps = psum.tile([C, P], f32, tag="xT")
                    nc.tensor.transpose(
                        out=xT_ps[:], in_=xt[:, g, :], identity=ident[:]
                    )
                    xT = pool.tile([C, P], f32, tag="xTs")
                    nc.vector.tensor_copy(out=xT[:], in_=xT_ps[:])
                    h_ps = psum.tile([P, 2 * C], f32, tag="h")
                    nc.tensor.matmul(
                        out=h_ps[:], lhsT=xT[:], rhs=wcat[:], start=True, stop=True
                    )
                    nc.scalar.copy(out=g_sup[:, g, :], in_=h_ps[:, C : 2 * C])
                    nc.vector.tensor_add(
                        out=a_sup[:, g, :], in0=h_ps[:, 0:C], in1=bb_sb[:]
                    )
                nc.sync.dma_start(
                    out=gout[rows, :].rearrange("(g p) c -> p g c", p=P), in_=g_sup[:]
                )
                nc.sync.dma_start(
                    out=aout[rows, :].rearrange("(g p) c -> p g c", p=P), in_=a_sup[:]
                )
    nc.compile()
    return nc


def _build_gather():
    import concourse.bacc as bacc
    import concourse.mybir as mybir
    from concourse.tile import TileContext

    nc = bacc.Bacc(
        "TRN2", target_bir_lowering=False, debug=False, num_swdge_queues=4
    )
    f32 = mybir.dt.float32
    i16 = mybir.dt.int16
    glo = nc.dram_tensor("glo", [SPLIT + 1, C], f32, kind="ExternalInput")
    ghi = nc.dram_tensor("ghi", [N_NODES - SPLIT + 1, C], f32, kind="ExternalInput")
    ilo = nc.dram_tensor("ilo", [NCHUNKS * P, NI // 16], i16, kind="ExternalInput")
    ihi = nc.dram_tensor("ihi", [NCHUNKS * P, NI // 16], i16, kind="ExternalInput")
    ash = nc.dram_tensor("ash", [NSH_PAD, C], f32, kind="ExternalInput")
    osh = nc.dram_tensor("osh", [NSH_PAD, C], f32, kind="ExternalOutput")

    with TileContext(nc) as tc:
        with (
            tc.tile_pool(name="sbuf", bufs=3) as pool,
            tc.tile_pool(name="gat", bufs=2) as gpool,
        ):
            for ch in range(NCHUNKS):
                irows = slice(ch * P, (ch + 1) * P)
                ncols = slice(ch * CHUNK, (ch + 1) * CHUNK)
                ilo_sb = pool.tile([P, NI // 16], i16, tag="ilo")
                nc.sync.dma_start(out=ilo_sb[:], in_=ilo[irows, :])
                ihi_sb = pool.tile([P, NI // 16], i16, tag="ihi")
                nc.sync.dma_start(out=ihi_sb[:], in_=ihi[irows, :])
                # dest layout [128, TCH*DEG slots, 64]: position j lands at
                # partition j%128, slot j//128
                dlo = gpool.tile([P, TCH * DEG, C], f32, tag="dlo")
                dhi = gpool.tile([P, TCH * DEG, C], f32, tag="dhi")
                nc.gpsimd.dma_gather(
                    out_ap=dlo[:],
                    in_ap=glo[:],
                    idxs_ap=ilo_sb[:],
                    num_idxs=NI,
                    num_idxs_reg=NI,
                    elem_size=C,
                    transpose=False,
                    queue_num=0,
                    single_packet=False,
                )
                nc.gpsimd.dma_gather(
                    out_ap=dhi[:],
                    in_ap=ghi[:],
                    idxs_ap=ihi_sb[:],
                    num_idxs=NI,
                    num_idxs_reg=NI,
                    elem_size=C,
                    transpose=False,
                    queue_num=0,
                    single_packet=False,
                )
                dm = gpool.tile([P, TCH * DEG, C], f32, tag="dm")
                nc.vector.tensor_max(out=dm[:], in0=dlo[:], in1=dhi[:])
                m_sb = pool.tile([P, TCH, C], f32, tag="m")
                nc.vector.tensor_reduce(
                    out=m_sb[:],
                    in_=dm[:].rearrange("p (t k) c -> p t c k", k=DEG),
                    axis=mybir.AxisListType.X,
                    op=mybir.AluOpType.max,
                )
                a_sb = pool.tile([P, TCH, C], f32, tag="a")
                nc.sync.dma_start(
                    out=a_sb[:], in_=ash[ncols, :].rearrange("(t p) c -> p t c", p=P)
                )
                s_sb = pool.tile([P, TCH, C], f32, tag="s")
                nc.vector.tensor_add(out=s_sb[:], in0=m_sb[:], in1=a_sb[:])
                o_sb = pool.tile([P, TCH, C], f32, tag="o")
                nc.scalar.activation(
                    out=o_sb[:], in_=s_sb[:], func=mybir.ActivationFunctionType.Relu
                )
                nc.sync.dma_start(
                    out=osh[ncols, :].rearrange("(t p) c -> p t c", p=P), in_=o_sb[:]
                )
    nc.compile()
    return nc


def _make_indices(src_pad):
    """src_pad: [NSH_PAD, DEG] int64 node ids (pad rows = -1).
    Returns (ilo, ihi) int16 arrays [NCHUNKS*128, NI//16] in dma_gather's
    index layout. Position j of chunk ch covers (node n_c=j%128 + 128*(j//128
    // DEG)... ) -- specifically j = (DEG*(n_c//128)+k)*128 + (n_c%128)."""
    out_lo = np.empty((NCHUNKS, P, NI // 16), dtype=np.int16)
    out_hi = np.empty((NCHUNKS, P, NI // 16), dtype=np.int16)
    s = src_pad.reshape(NCHUNKS, TCH, P, DEG)
    # flat position order within a chunk: j = ((t*DEG + k) * P + p)
    flat = np.transpose(s, (0, 1, 3, 2)).reshape(NCHUNKS, NI)  # [ch, (t k p)]
    lo = np.where((flat >= 0) & (flat < SPLIT), flat + 1, 0).astype(np.int16)
    hi = np.where(flat >= SPLIT, flat - SPLIT + 1, 0).astype(np.int16)
    for arr, out in ((lo, out_lo), (hi, out_hi)):
        # index layout: position j -> [j%16, j//16], replicated 8x down parts
        a = arr.reshape(NCHUNKS, NI // 16, 16)
        a = np.swapaxes(a, 1, 2)  # [ch, 16, s]
        out[:] = np.tile(a, (1, 8, 1))
    return (
        np.ascontiguousarray(out_lo.reshape(NCHUNKS * P, NI // 16)),
        np.ascontiguousarray(out_hi.reshape(NCHUNKS * P, NI // 16)),
    )


def _numpy_fallback(x, edge_index, W, b):
    src, dst = edge_index[0], edge_index[1]
    V1 = W[:, :C] - W[:, C:]
    V2 = W[:, C:]
    A = x @ V1.T + b
    g = x @ V2.T
    out = np.full((x.shape[0], C), -np.inf, dtype=np.float32)
    msg = np.maximum(A[dst] + g[src], 0.0)
    np.maximum.at(out, dst, msg)
    return np.where(np.isneginf(out), 0.0, out).astype(np.float32)


def kernel(x, edge_index, edge_attr, W, b):
    from concourse.bass_utils import run_bass_kernel_spmd

    x = np.ascontiguousarray(x, dtype=np.float32)
    edge_index = np.ascontiguousarray(edge_index, dtype=np.int32)
    W = np.ascontiguousarray(W, dtype=np.float32)
    b = np.ascontiguousarray(b, dtype=np.float32)

    expected_dst = np.repeat(np.arange(N_NODES, dtype=np.int32), DEG)
    if (
        x.shape != (N_NODES, C)
        or edge_index.shape != (2, N_NODES * DEG)
        or not np.array_equal(edge_index[1], expected_dst)
    ):
        return _numpy_fallback(x, edge_index, W, b)

    if "dense" not in _cache:
        _cache["dense"] = _build_dense()
    if "gather" not in _cache:
        _cache["gather"] = _build_gather()

    # ---- Launch 1: node-parallel dense phase ----
    bb = np.ascontiguousarray(np.broadcast_to(b, (P, C)))
    in1 = []
    for c in range(N_CORES):
        xs = np.zeros((NSH_PAD, C), dtype=np.float32)
        xs[:NSH] = x[c * NSH : (c + 1) * NSH]
        in1.append({"xs": xs, "w": W, "bb": bb})
    r1 = run_bass_kernel_spmd(_cache["dense"], in1, core_ids=list(range(N_CORES)))

    g_full = np.concatenate(
        [r1.results[c]["gout"][:NSH] for c in range(N_CORES)], axis=0
    )
    sent_row = np.full((1, C), SENT, dtype=np.float32)
    glo = np.ascontiguousarray(np.concatenate([sent_row, g_full[:SPLIT]], axis=0))
    ghi = np.ascontiguousarray(np.concatenate([sent_row, g_full[SPLIT:]], axis=0))

    # ---- Launch 2: edge-parallel gather + segment max ----
    src = edge_index[0]
    in2 = []
    for c in range(N_CORES):
        s = np.full((NSH_PAD, DEG), -1, dtype=np.int64)
        s[:NSH] = src[c * NSH * DEG : (c + 1) * NSH * DEG].reshape(NSH, DEG)
        ilo, ihi = _make_indices(s)
        in2.append(
            {"glo": glo, "ghi": ghi, "ilo": ilo, "ihi": ihi, "ash": r1.results[c]["aout"]}
        )
    r2 = run_bass_kernel_spmd(_cache["gather"], in2, core_ids=list(range(N_CORES)))

    out = np.concatenate(
        [r2.results[c]["osh"][:NSH] for c in range(N_CORES)], axis=0
    ).astype(np.float32)
    _cache["last_results"] = (r1, r2)
    return out


# revision 21
# speedup vs baseline: 15393.4240x; 15393.4240x over previous
"""EdgeConv (PyG, aggr='max') Trainium2 kernel, 8-core SPMD.

Math: out_i = max_{e: dst(e)=i} relu(x_i @ W1.T + (x_src(e) - x_i) @ W2.T + b)
with W = [W1 | W2].  Rewriting:
    msg_e = relu(A_i + g_src(e)),  A = x @ (W1-W2).T + b,  g = x @ W2.T
Since A_i is constant within segment i and relu is monotone:
    out_i = relu(A_i + max_e g_src(e))
The reference's dst is repeat(arange(N), DEG) (fixed-degree kNN-style graph),
so segments are 16 consecutive edges; segment-max becomes a grouped reduce.

Two SPMD launches on 8 cores:
  L1 (node-parallel): per-core 6250-node shard computes A (row-major f32)
     and g (row-major bf16) via PE transpose of x tiles + one matmul.
  L2 (edge-parallel): per-core 100k-edge shard bulk-gathers 256B bf16
     row-PAIRS [g_{2r} | g_{2r+1}] by src>>1 with non-transpose dma_gather
     (one descriptor per edge — half the HBM bytes of fp32 pairs; src>>1 <=
     24999 fits int16 with no table split; pad positions read the sentinel
     pair-row NPAIR = -3e38). The parity half-select uses a host-precomputed
     uint8 mask via copy_predicated (plain copy on ACT, predicated overwrite
     on DVE). A host-side edge permutation lands node n's 16 slots at
     partition n%128, slots 16*(n//128)+k, so the segment max is a free-dim
     grouped reduce emitting f32 row-major directly.
"""

import numpy as np

N_NODES = 50000
DEG = 16
C = 64
N_CORES = 8
NSH = N_NODES // N_CORES  # 6250 nodes per core
P = 128
TCH = 2  # node-tiles per L2 chunk
CHUNK = TCH * P  # 256 nodes per L2 chunk
NSH_PAD = 6400  # 25 chunks * 256; 50 tiles * 128
NT = NSH_PAD // P  # 50
NCHUNKS = NSH_PAD // CHUNK  # 25
NI = CHUNK * DEG  # 4096 gather positions per chunk
NPAIR = N_NODES // 2  # 512B row-pairs in the gather table
SENT = -3.0e38

_cache = {}


def _build_dense():
    import concourse.bacc as bacc
    import concourse.mybir as mybir
    from concourse.tile import TileContext
    from concourse.masks import make_identity

    nc = bacc.Bacc("TRN2", target_bir_lowering=False, debug=False)
    f32 = mybir.dt.float32
    bf16 = mybir.dt.bfloat16
    xs = nc.dram_tensor("xs", [NSH_PAD, C], f32, kind="ExternalInput")
    w = nc.dram_tensor("w", [C, 2 * C], f32, kind="ExternalInput")
    bb = nc.dram_tensor("bb", [P, C], f32, kind="ExternalInput")
    gout = nc.dram_tensor("gout", [NSH_PAD, C], bf16, kind="ExternalOutput")
    aout = nc.dram_tensor("aout", [NSH_PAD, C], f32, kind="ExternalOutput")

    with TileContext(nc) as tc:
        with (
            tc.tile_pool(name="const", bufs=1) as cpool,
            tc.tile_pool(name="sbuf", bufs=4) as pool,
            tc.tile_pool(name="psum", bufs=2, space="PSUM") as psum,
        ):
            ident = cpool.tile([P, P], f32)
            make_identity(nc, ident[:])
            w_sb = cpool.tile([C, 2 * C], f32)
            nc.sync.dma_start(out=w_sb[:], in_=w[:])
            bb_sb = cpool.tile([P, C], f32)
            nc.sync.dma_start(out=bb_sb[:], in_=bb[:])

            # wcat = [V1.T | V2.T] ([64, 128]): V1=W1-W2, V2=W2.
            t1 = psum.tile([C, C], f32, tag="wt")
            t2 = psum.tile([C, C], f32, tag="wt")
            nc.tensor.transpose(out=t1[:], in_=w_sb[:, 0:C], identity=ident[:C, :C])
            nc.tensor.transpose(out=t2[:], in_=w_sb[:, C : 2 * C], identity=ident[:C, :C])
            wcat = cpool.tile([C, 2 * C], f32)
            nc.vector.tensor_copy(out=wcat[:, C : 2 * C], in_=t2[:])
            nc.vector.tensor_sub(out=wcat[:, 0:C], in0=t1[:], in1=wcat[:, C : 2 * C])

            G = 5  # node-tiles per supertile (NT == 50 == 10 * G)
            for st in range(NT // G):
                rows = slice(st * G * P, (st + 1) * G * P)
                xt = pool.tile([P, G, C], f32, tag="xt")
                nc.sync.dma_start(
                    out=xt[:], in_=xs[rows, :].rearrange("(g p) c -> p g c", p=P)
                )
                g_sup = pool.tile([P, G, C], bf16, tag="g")
                a_sup = pool.tile([P, G, C], f32, tag="a")
                for g in range(G):
                    xT_ps = psum.tile([C, P], f32, tag="xT")
                    nc.tensor.transpose(
                        out=xT_ps[:], in_=xt[:, g, :], identity=ident[:]
                    )
                    xT = pool.tile([C, P], f32, tag="xTs")
                    nc.vector.tensor_copy(out=xT[:], in_=xT_ps[:])
                    h_ps = psum.tile([P, 2 * C], f32, tag="h")
                    nc.tensor.matmul(
                        out=h_ps[:], lhsT=xT[:], rhs=wcat[:], start=True, stop=True
                    )
                    nc.scalar.copy(out=g_sup[:, g, :], in_=h_ps[:, C : 2 * C])
                    nc.vector.tensor_add(
                        out=a_sup[:, g, :], in0=h_ps[:, 0:C], in1=bb_sb[:]
                    )
                nc.sync.dma_start(
                    out=gout[rows, :].rearrange("(g p) c -> p g c", p=P), in_=g_sup[:]
                )
                nc.sync.dma_start(
                    out=aout[rows, :].rearrange("(g p) c -> p g c", p=P), in_=a_sup[:]
                )
    nc.compile()
    return nc


def _build_gather():
    import concourse.bacc as bacc
    import concourse.mybir as mybir
    from concourse.tile import TileContext

    nc = bacc.Bacc("TRN2", target_bir_lowering=False, debug=False)
    f32 = mybir.dt.float32
    bf16 = mybir.dt.bfloat16
    i16 = mybir.dt.int16
    # pair table: row r = [g_{2r} | g_{2r+1}] (512B); row NPAIR = sentinel
    gpair = nc.dram_tensor("gpair", [NPAIR + 1, 2 * C], bf16, kind="ExternalInput")
    idx = nc.dram_tensor("idx", [NCHUNKS * P, NI // 16], i16, kind="ExternalInput")
    msk = nc.dram_tensor("msk", [NCHUNKS * P, TCH * DEG], mybir.dt.uint8, kind="ExternalInput")
    ash = nc.dram_tensor("ash", [NSH_PAD, C], f32, kind="ExternalInput")
    osh = nc.dram_tensor("osh", [NSH_PAD, C], f32, kind="ExternalOutput")

    with TileContext(nc) as tc:
        with (
            tc.tile_pool(name="sbuf", bufs=4) as pool,
            tc.tile_pool(name="gat", bufs=4) as gpool,
        ):
            for ch in range(NCHUNKS):
                irows = slice(ch * P, (ch + 1) * P)
                ncols = slice(ch * CHUNK, (ch + 1) * CHUNK)
                idx_sb = pool.tile([P, NI // 16], i16, tag="idx")
                nc.sync.dma_start(out=idx_sb[:], in_=idx[irows, :])
                msk_sb = pool.tile([P, TCH * DEG], mybir.dt.uint8, tag="msk")
                nc.sync.dma_start(out=msk_sb[:], in_=msk[irows, :])
                # position j lands at partition j%128, slot j//128; each slot
                # holds a 512B row-pair [even | odd]
                gath = gpool.tile([P, TCH * DEG, 2 * C], bf16, tag="gath")
                nc.gpsimd.dma_gather(
                    out_ap=gath[:],
                    in_ap=gpair[:],
                    idxs_ap=idx_sb[:],
                    num_idxs=NI,
                    num_idxs_reg=NI,
                    elem_size=2 * C,
                    transpose=False,
                    queue_num=0,
                    single_packet=False,
                )
                # select the parity half: even by default (ACT), odd where mask=1
                sel = gpool.tile([P, TCH * DEG, C], bf16, tag="sel")
                nc.scalar.copy(out=sel[:], in_=gath[:, :, 0:C])
                nc.vector.copy_predicated(
                    out=sel[:],
                    mask=msk_sb[:].to_broadcast([P, TCH * DEG, C]),
                    data=gath[:, :, C : 2 * C],
                )
                m_sb = pool.tile([P, TCH, C], f32, tag="m")
                nc.vector.tensor_reduce(
                    out=m_sb[:],
                    in_=sel[:].rearrange("p (t k) c -> p t c k", k=DEG),
                    axis=mybir.AxisListType.X,
                    op=mybir.AluOpType.max,
                )
                a_sb = pool.tile([P, TCH, C], f32, tag="a")
                nc.sync.dma_start(
                    out=a_sb[:], in_=ash[ncols, :].rearrange("(t p) c -> p t c", p=P)
                )
                s_sb = pool.tile([P, TCH, C], f32, tag="s")
                nc.vector.tensor_add(out=s_sb[:], in0=m_sb[:], in1=a_sb[:])
                o_sb = pool.tile([P, TCH, C], f32, tag="o")
                nc.scalar.activation(
                    out=o_sb[:], in_=s_sb[:], func=mybir.ActivationFunctionType.Relu
                )
                nc.sync.dma_start(
                    out=osh[ncols, :].rearrange("(t p) c -> p t c", p=P), in_=o_sb[:]
                )
    nc.compile()
    return nc


def _make_indices(src_pad):
    """src_pad: [NSH_PAD, DEG] int64 node ids (pad rows = -1).
    Returns (idx, msk): pair-row indices (src>>1, sentinel NPAIR for pads) in
    dma_gather's index layout, and the odd-parity mask in dest layout
    [128, slots]. Position j of chunk ch covers node n_c = j%128 + 128*(j//128
    // DEG) ... specifically j = (DEG*(n_c//128)+k)*128 + (n_c%128)."""
    s = src_pad.reshape(NCHUNKS, TCH, P, DEG)
    flat = np.transpose(s, (0, 1, 3, 2)).reshape(NCHUNKS, NI)  # [ch, (t k p)]
    pidx = np.where(flat >= 0, flat >> 1, NPAIR).astype(np.int16)
    par = np.where(flat >= 0, flat & 1, 0).astype(np.uint8)
    # index layout: position j -> [j%16, j//16], replicated 8x down partitions
    a = np.swapaxes(pidx.reshape(NCHUNKS, NI // 16, 16), 1, 2)
    idx = np.ascontiguousarray(
        np.tile(a, (1, 8, 1)).reshape(NCHUNKS * P, NI // 16)
    )
    # mask layout: dest [partition j%128, slot j//128]
    m = np.swapaxes(par.reshape(NCHUNKS, TCH * DEG, P), 1, 2)
    msk = np.ascontiguousarray(m.reshape(NCHUNKS * P, TCH * DEG))
    return idx, msk


def _numpy_fallback(x, edge_index, W, b):
    src, dst = edge_index[0], edge_index[1]
    V1 = W[:, :C] - W[:, C:]
    V2 = W[:, C:]
    A = x @ V1.T + b
    g = x @ V2.T
    out = np.full((x.shape[0], C), -np.inf, dtype=np.float32)
    msg = np.maximum(A[dst] + g[src], 0.0)
    np.maximum.at(out, dst, msg)
    return np.where(np.isneginf(out), 0.0, out).astype(np.float32)


def kernel(x, edge_index, edge_attr, W, b):
    from concourse.bass_utils import run_bass_kernel_spmd

    x = np.ascontiguousarray(x, dtype=np.float32)
    edge_index = np.ascontiguousarray(edge_index, dtype=np.int32)
    W = np.ascontiguousarray(W, dtype=np.float32)
    b = np.ascontiguousarray(b, dtype=np.float32)

    expected_dst = np.repeat(np.arange(N_NODES, dtype=np.int32), DEG)
    if (
        x.shape != (N_NODES, C)
        or edge_index.shape != (2, N_NODES * DEG)
        or not np.array_equal(edge_index[1], expected_dst)
    ):
        return _numpy_fallback(x, edge_index, W, b)

    if "dense" not in _cache:
        _cache["dense"] = _build_dense()
    if "gather" not in _cache:
        _cache["gather"] = _build_gather()

    # ---- Launch 1: node-parallel dense phase ----
    bb = np.ascontiguousarray(np.broadcast_to(b, (P, C)))
    in1 = []
    for c in range(N_CORES):
        xs = np.zeros((NSH_PAD, C), dtype=np.float32)
        xs[:NSH] = x[c * NSH : (c + 1) * NSH]
        in1.append({"xs": xs, "w": W, "bb": bb})
    r1 = run_bass_kernel_spmd(_cache["dense"], in1, core_ids=list(range(N_CORES)))

    g_full = np.concatenate(
        [r1.results[c]["gout"][:NSH] for c in range(N_CORES)], axis=0
    )
    gpair = np.concatenate(
        [g_full.reshape(NPAIR, 2 * C), np.full((1, 2 * C), SENT, dtype=g_full.dtype)],
        axis=0,
    )
    gpair = np.ascontiguousarray(gpair)

    # ---- Launch 2: edge-parallel gather + segment max ----
    src = edge_index[0]
    in2 = []
    for c in range(N_CORES):
        s = np.full((NSH_PAD, DEG), -1, dtype=np.int64)
        s[:NSH] = src[c * NSH * DEG : (c + 1) * NSH * DEG].reshape(NSH, DEG)
        idx, msk = _make_indices(s)
        in2.append(
            {"gpair": gpair, "idx": idx, "msk": msk, "ash": r1.results[c]["aout"]}
        )
    r2 = run_bass_kernel_spmd(_cache["gather"], in2, core_ids=list(range(N_CORES)))

    out = np.concatenate(
        [r2.results[c]["osh"][:NSH] for c in range(N_CORES)], axis=0
    ).astype(np.float32)
    _cache["last_results"] = (r1, r2)
    return out

